# revision 6
# baseline (speedup 1.0000x reference)
"""LSTM (B=512, T=512, D=32, H=64) + sigmoid linear head on 8 NeuronCores.

Data-parallel over batch (64 per core); the T=512 recurrence runs locally
per core. Everything lives in transposed [feature, batch] layout so the
per-step matmul contracts over partitions.

Per-step math tricks (host-side weight preprocessing):
  - W = [W_hh | W_ih | b_ih+b_hh] stacked as lhsT [97, 256]; rhs is a
    [h_{t-1}; x_t; 1] column block, so one pair of matmuls produces all
    four gate pre-activations.
  - g-gate rows of W are pre-scaled by 2 => sigmoid(2g) comes out of the
    SAME sigmoid op as i/f/o (tanh(g) = 2*sigmoid(2g) - 1).
  - cell state is stored halved (c' = c/2):
        c' = f*c' + i*(sigmoid(2g) - 0.5)
    which is a single fused scalar_tensor_tensor plus a mul and an add,
    and tanh(c) = Tanh(2*c') uses the activation's free input scale.
  - h_t is written directly into the next step's rhs tile (also the hs
    output staging buffer), and a tiny per-step matmul accumulates the
    linear head [64,1] into a persistent PSUM bank, sigmoided once at
    the end.
"""

from contextlib import ExitStack

import numpy as np

import concourse.bacc as bacc
import concourse.bass as bass
import concourse.mybir as mybir
import concourse.tile as tile
from concourse.bass_utils import run_bass_kernel_spmd

B, T, D, H = 512, 512, 32, 64
NCORES = 8
BL = B // NCORES  # 64 batch per core
KD = H + D + 1  # 97: [h; x; ones]
TC = 64  # timesteps per chunk tile
F32 = mybir.dt.float32
AF = mybir.ActivationFunctionType
ALU = mybir.AluOpType


def build_program(t_steps: int = T):
    nch = t_steps // TC if t_steps >= TC else 1
    tcs = min(TC, t_steps)
    nc = bacc.Bacc()
    xa = nc.declare_dram_parameter("xa", [D + 1, t_steps, BL], F32, False)
    h0 = nc.declare_dram_parameter("h0", [H, BL], F32, False)
    c0h = nc.declare_dram_parameter("c0h", [H, BL], F32, False)
    wa = nc.declare_dram_parameter("wa", [KD, 2 * H], F32, False)
    wb = nc.declare_dram_parameter("wb", [KD, 2 * H], F32, False)
    wo = nc.declare_dram_parameter("wo", [H, 1], F32, False)
    bo = nc.declare_dram_parameter("bo", [BL, 1], F32, False)
    hs = nc.declare_dram_parameter("hs", [H, t_steps + 1, BL], F32, True)
    hd = nc.declare_dram_parameter("hd", [BL, t_steps], F32, True)

    xa_r = xa.rearrange("d t b -> d (t b)")
    hs_r = hs.rearrange("h t b -> h (t b)")

    with tile.TileContext(nc) as tc, ExitStack() as ctx:
        const_pool = ctx.enter_context(tc.tile_pool(name="const", bufs=1))
        rhs_pool = ctx.enter_context(tc.tile_pool(name="rhs", bufs=nch + 1))
        sig_pool = ctx.enter_context(tc.tile_pool(name="sig", bufs=4))
        small_pool = ctx.enter_context(tc.tile_pool(name="small", bufs=4))
        state_pool = ctx.enter_context(tc.tile_pool(name="state", bufs=4))
        gpsum_pool = ctx.enter_context(
            tc.tile_pool(name="gpsum", bufs=4, space="PSUM")
        )
        hpsum_pool = ctx.enter_context(
            tc.tile_pool(name="hpsum", bufs=1, space="PSUM")
        )

        wa_t = const_pool.tile([KD, 2 * H], F32, tag="wa")
        nc.sync.dma_start(wa_t[:], wa[:])
        wb_t = const_pool.tile([KD, 2 * H], F32, tag="wb")
        nc.sync.dma_start(wb_t[:], wb[:])
        wo_t = const_pool.tile([H, 1], F32, tag="wo")
        nc.sync.dma_start(wo_t[:], wo[:])
        bo_t = const_pool.tile([BL, 1], F32, tag="bo")
        nc.sync.dma_start(bo_t[:], bo[:])

        head_ps = hpsum_pool.tile([BL, t_steps], F32, tag="head")

        def alloc_chunk(c):
            t_ = rhs_pool.tile([KD, tcs * BL], F32, tag="chunk")
            nc.sync.dma_start(
                t_[H:KD, :], xa_r[:, c * tcs * BL : (c + 1) * tcs * BL]
            )
            return t_

        chunk_tiles = [alloc_chunk(0)]
        nc.sync.dma_start(chunk_tiles[0][0:H, 0:BL], h0[:])
        stub = rhs_pool.tile([KD, BL], F32, tag="stub")

        # cell state lives on partitions 64:128 so every 2-input DVE op has
        # both SBUF operands at the same base partition (HW verifier rule)
        c_state = state_pool.tile([128, BL], F32, tag="c")
        nc.sync.dma_start(c_state[H:128, :], c0h[:])

        for t in range(t_steps):
            c = t // tcs
            j = t % tcs
            cur = chunk_tiles[c]
            if j == 0 and c + 1 < nch:
                chunk_tiles.append(alloc_chunk(c + 1))
            if t + 1 < t_steps:
                nxt = chunk_tiles[(t + 1) // tcs]
                jn = (t + 1) % tcs
                hdst = nxt[0:H, jn * BL : (jn + 1) * BL]
            else:
                hdst = stub[0:H, 0:BL]

            rhs_ap = cur[0:KD, j * BL : (j + 1) * BL]
            g_ps = gpsum_pool.tile([128, 2 * BL], F32, tag="g")
            nc.tensor.matmul(g_ps[:, 0:BL], wa_t[:], rhs_ap, start=True, stop=True)
            nc.tensor.matmul(
                g_ps[:, BL : 2 * BL], wb_t[:], rhs_ap, start=True, stop=True
            )
            # s: cols 0:BL = [sig_i; sig_f], cols BL:2BL = [sig_2g; sig_o]
            s_t = sig_pool.tile([128, 2 * BL], F32, tag="s")
            nc.scalar.activation(s_t[:], g_ps[:], AF.Sigmoid)

            r_t = small_pool.tile([128, BL], F32, tag="r")
            nc.vector.tensor_mul(
                r_t[H:128, :], s_t[H:128, 0:BL], c_state[H:128, :]
            )  # f*c'
            q_t = small_pool.tile([128, BL], F32, tag="q")
            nc.vector.scalar_tensor_tensor(
                q_t[H:128, :],
                s_t[0:H, BL : 2 * BL],
                0.5,
                s_t[0:H, 0:BL],
                ALU.subtract,
                ALU.mult,
            )  # (sig_2g - 0.5) * i
            c_new = state_pool.tile([128, BL], F32, tag="c")
            nc.vector.tensor_add(c_new[H:128, :], r_t[H:128, :], q_t[H:128, :])
            c_state = c_new

            tc_t = small_pool.tile([128, BL], F32, tag="tc")
            nc.scalar.activation(tc_t[H:128, :], c_new[H:128, :], AF.Tanh, scale=2.0)
            nc.vector.tensor_mul(
                hdst, s_t[H:128, BL : 2 * BL], tc_t[H:128, :]
            )  # o*tanh

            nc.tensor.matmul(
                head_ps[:, t : t + 1], hdst, wo_t[:], start=True, stop=True
            )

            if j == tcs - 1:
                nc.sync.dma_start(
                    hs_r[:, c * tcs * BL : (c + 1) * tcs * BL], cur[0:H, :]
                )

        nc.sync.dma_start(hs_r[:, t_steps * BL : (t_steps + 1) * BL], stub[0:H, :])
        head_sb = const_pool.tile([BL, t_steps], F32, tag="headsb")
        nc.scalar.activation(
            head_sb[:], head_ps[:], AF.Sigmoid, bias=bo_t[:, 0:1]
        )
        nc.sync.dma_start(hd[:, :], head_sb[:])

    nc.finalize()
    return nc


def make_in_maps(x, h_0, c_0, W_ih, W_hh, b_ih, b_hh, W_out, b_out):
    x = np.asarray(x, np.float32)
    t_steps = x.shape[1]
    Wc = np.concatenate(
        [
            np.asarray(W_hh, np.float32),
            np.asarray(W_ih, np.float32),
            (np.asarray(b_ih, np.float32) + np.asarray(b_hh, np.float32))[:, None],
        ],
        axis=1,
    )  # [4H, 97]
    Wc = Wc.copy()
    Wc[2 * H : 3 * H, :] *= 2.0  # g rows: sigmoid(2g) trick
    wa = np.ascontiguousarray(Wc[0 : 2 * H, :].T)  # [97, 128] (i, f)
    wb = np.ascontiguousarray(Wc[2 * H : 4 * H, :].T)  # [97, 128] (2g, o)
    wo = np.ascontiguousarray(np.asarray(W_out, np.float32).T)  # [H, 1]
    bscalar = float(np.asarray(b_out, np.float32).reshape(-1)[0])

    xa_full = np.concatenate(
        [x, np.ones((x.shape[0], t_steps, 1), np.float32)], axis=2
    )  # [B, T, 33]
    xa_full = np.ascontiguousarray(xa_full.transpose(2, 1, 0))  # [33, T, B]
    h0_full = np.ascontiguousarray(np.asarray(h_0, np.float32)[0].T)  # [H, B]
    c0_full = np.ascontiguousarray(np.asarray(c_0, np.float32)[0].T) * 0.5

    in_maps = []
    for k in range(NCORES):
        sl = slice(k * BL, (k + 1) * BL)
        in_maps.append(
            {
                "xa": np.ascontiguousarray(xa_full[:, :, sl]),
                "h0": np.ascontiguousarray(h0_full[:, sl]),
                "c0h": np.ascontiguousarray(c0_full[:, sl]),
                "wa": wa,
                "wb": wb,
                "wo": wo,
                "bo": np.full((BL, 1), bscalar, np.float32),
            }
        )
    return in_maps


def assemble_outputs(results, t_steps: int = T):
    bsz = NCORES * BL
    hs_out = np.empty((bsz, t_steps, H), np.float32)
    out = np.empty((bsz, t_steps, 1), np.float32)
    for k in range(NCORES):
        hs_k = np.asarray(results[k]["hs"])  # [H, T+1, BL]
        hs_out[k * BL : (k + 1) * BL] = hs_k[:, 1:, :].transpose(2, 1, 0)
        out[k * BL : (k + 1) * BL, :, 0] = np.asarray(results[k]["hd"])  # [BL, T]
    return out, hs_out


def kernel(x, h_0, c_0, W_ih, W_hh, b_ih, b_hh, W_out, b_out):
    in_maps = make_in_maps(x, h_0, c_0, W_ih, W_hh, b_ih, b_hh, W_out, b_out)
    nc = build_program(T)
    res = run_bass_kernel_spmd(nc, in_maps, list(range(NCORES))).results
    return assemble_outputs(res, T)


if __name__ == "__main__":
    nc = build_program(T)
    print("build ok")
